# revision 4
# baseline (speedup 1.0000x reference)
"""MoE balancing-loss kernel for Trainium2 (8 NeuronCores, data-parallel over tokens).

Problem: router_logits [32, 16384, 64] f32 ->
    loss = 0.01 * sum_l (E/(T*K)) * sum_e counts[l,e] * mean_t(softmax(logits)[l,t,e])
where counts[l,e] = #tokens whose top-8 (by softmax == by logits) includes expert e.

Sharding: tokens (dim 1) split across 8 cores, 2048 tokens/core. Each core
computes partial counts[l,e] and partial sum_t softmax[l,t,e]; host reduces the
tiny [32,64] partials and forms the loss (the global-average all-reduce).

Per-core layout (per layer): one SBUF tile [128 partitions x 1024] f32 where
partition p holds 16 consecutive tokens (slots j=0..15) of 64 logits each.
  ACT : e = exp(x) -> bf16 (no max-subtract needed: |x| <~ 6)
  DVE : 16x max8 (8 largest per token -> threshold = 8th), segmented
        reduce_sum for softmax denominators s[p,j], reciprocal r = 1/s
  POOL: mask = (x >= theta) as bf16 via broadcast tensor_tensor; r -> bf16 cast
  PE  : per slot j: rwsum[1,64] += r_j^T @ e_j ; counts[1,64] += ones^T @ mask_j
        (PSUM-accumulated over the 16 slots; K-tiling of the token contraction)
  out : per layer [1,128] = [rwsum | counts] -> staged -> one DMA out
"""

import numpy as np

L, T, E = 32, 16384, 64
K = 8
NCORES = 8
TC = T // NCORES          # 2048 tokens per core
P = 128                   # partitions
J = TC // P               # 16 token slots per partition
LOSS_WEIGHT = 0.01

_cached = {}


def _build():
    import concourse.bacc as bacc
    import concourse.mybir as mybir
    from concourse.tile import TileContext

    f32 = mybir.dt.float32
    bf16 = mybir.dt.bfloat16
    Alu = mybir.AluOpType

    nc = bacc.Bacc(trn_type="TRN2")
    x = nc.dram_tensor("x", [L, P, J * E], f32, kind="ExternalInput")
    out = nc.dram_tensor("out", [1, L * 2 * E], f32, kind="ExternalOutput")

    with TileContext(nc) as tc:
        with (
            tc.tile_pool(name="const", bufs=1) as cpool,
            tc.tile_pool(name="work", bufs=3) as pool,
            tc.tile_pool(name="ps", bufs=4, space="PSUM") as ppool,
        ):
            ones_bf = cpool.tile([P, 1], bf16)
            nc.vector.memset(ones_bf[:], 1.0)
            stage = cpool.tile([1, L * 2 * E], f32)

            for l in range(L):
                x_t = pool.tile([P, J * E], f32, tag="x")
                nc.sync.dma_start(x_t[:], x[l])
                x3d = x_t[:].rearrange("p (j e) -> p j e", e=E)

                # exp -> bf16
                e_t = pool.tile([P, J * E], bf16, tag="e")
                nc.scalar.activation(
                    e_t[:], x_t[:], mybir.ActivationFunctionType.Exp
                )

                # softmax denominators per token: s[p, j] = sum_e e[p, j, e]
                s_t = pool.tile([P, J], f32, tag="s")
                nc.vector.reduce_sum(
                    s_t[:],
                    e_t[:].rearrange("p (j e) -> p j e", e=E),
                    axis=mybir.AxisListType.X,
                )
                r_t = pool.tile([P, J], f32, tag="r")
                nc.vector.reciprocal(r_t[:], s_t[:])
                r_bf = pool.tile([P, J], bf16, tag="rbf")
                nc.scalar.copy(r_bf[:], r_t[:])

                # top-8 per token (f32, exact threshold = 8th largest)
                th_t = pool.tile([P, J * 8], f32, tag="th")
                for j in range(J):
                    nc.vector.max(
                        out=th_t[:, j * 8 : (j + 1) * 8],
                        in_=x_t[:, j * E : (j + 1) * E],
                    )

                # mask = (x >= theta_token) -> bf16, per-slot tensor_scalar on DVE
                # (theta is a per-partition scalar [P,1] for each slot; fp32
                # single-src SBUF tensor_scalar runs in 2x_2P mode)
                mask_t = pool.tile([P, J * E], bf16, tag="mask")
                for j in range(J):
                    nc.vector.tensor_scalar(
                        mask_t[:, j * E : (j + 1) * E],
                        x_t[:, j * E : (j + 1) * E],
                        th_t[:, j * 8 + 7 : j * 8 + 8],
                        None,
                        Alu.is_ge,
                    )

                # PE: accumulate rwsum and counts over the 16 slots
                acc = ppool.tile([1, 2 * E], f32, tag="acc")
                for j in range(J):
                    nc.tensor.matmul(
                        acc[0:1, 0:E],
                        r_bf[:, j : j + 1],
                        e_t[:, j * E : (j + 1) * E],
                        start=(j == 0),
                        stop=(j == J - 1),
                    )
                for j in range(J):
                    nc.tensor.matmul(
                        acc[0:1, E : 2 * E],
                        ones_bf[:, 0:1],
                        mask_t[:, j * E : (j + 1) * E],
                        start=(j == 0),
                        stop=(j == J - 1),
                    )

                # PSUM -> SBUF staging (ACT)
                nc.scalar.copy(
                    stage[0:1, l * 2 * E : (l + 1) * 2 * E], acc[0:1, :]
                )

            nc.sync.dma_start(out[:], stage[:])

    nc.finalize()
    return nc


def _get_nc():
    if "nc" not in _cached:
        _cached["nc"] = _build()
    return _cached["nc"]


def kernel(router_logits, n_routed_experts=E, num_experts_per_tok=K):
    from concourse.bass_utils import run_bass_kernel_spmd

    xl = np.asarray(router_logits, dtype=np.float32)
    assert xl.shape == (L, T, E), xl.shape
    assert int(n_routed_experts) == E and int(num_experts_per_tok) == K

    nc = _get_nc()
    in_maps = []
    for c in range(NCORES):
        sl = np.ascontiguousarray(xl[:, c * TC : (c + 1) * TC, :])
        in_maps.append({"x": sl.reshape(L, P, J * E)})

    res = run_bass_kernel_spmd(nc, in_maps, core_ids=list(range(NCORES)))

    rwsum = np.zeros((L, E), np.float64)
    counts = np.zeros((L, E), np.float64)
    for c in range(NCORES):
        o = np.asarray(res.results[c]["out"]).reshape(L, 2 * E)
        rwsum += o[:, :E]
        counts += o[:, E:]

    scale = E / (T * K)
    rw_mean = rwsum / T
    loss = (scale * (counts * rw_mean).sum(-1)).sum() * LOSS_WEIGHT
    return np.float32(loss)
